# revision 19
# baseline (speedup 1.0000x reference)
"""DeepSeek MLA head — Trainium2 Bass kernel, 8 NeuronCores. v2.

Sharding: 8 cores = 2 batches x 4 cores. Each core owns one batch and 4 of
the 16 heads; latent projections replicated within each batch's 4 cores;
each core emits a partial o_proj output [S, HID] (f16) which the host sums.

v2 vs v1 (652us):
- Supertile-merged schedule: P1 (latents/qkv), P2 (attention qs=st), P3
  (o-proj token chunks of st) interleave per supertile so TensorE always has
  independent matmul work and HAM stays warm.
- DVE `reciprocal` (3.3us, single-lane) replaced by reciprocal_approx_fast.
- Broadcast matmuls + ScalarE copies replaced by gpsimd.partition_broadcast.
- RMSNorm of q folded into the rope cos/sin tables (per-supertile), so q_b
  matmuls never wait on the norm chain.
- PSUM->SBUF copies on nc.any (scheduler balances ACT/DVE).
- Output stored f16 (halved DMA), host accumulates in f32.
"""
import sys
import types

sys.path.insert(0, "/opt/trn_rl_repo")

import numpy as np

B, S, HID, NH = 2, 2048, 2048, 16
ROPE, NOPE, VDIM = 64, 64, 128
QHEAD, QLORA, KVLORA = 128, 682, 256
THETA = 128000.0
SCALE = 1.0 / float(np.sqrt(128.0))
EPS = 1e-6
HPC = 4              # heads per core
NCORES = 8
QCH = [128, 128, 128, 128, 128, 42]   # qlora partition chunks
NST = 4              # 512-token supertiles per batch
STW = 512

_PROGRAM = None


def _ensure_axon_hooks_shim():
    if "antenv.axon_hooks" in sys.modules:
        return
    try:
        from trn_agent_boot.trn_boot import _ntff_profile_via_ctypes
        hook = _ntff_profile_via_ctypes("/opt/axon/libaxon_pjrt.so")
    except Exception:
        hook = None
    m = types.ModuleType("antenv.axon_hooks")
    m.get_axon_ntff_profile_hook = lambda: hook
    m.set_axon_ntff_profile_hook = lambda h: None
    sys.modules["antenv.axon_hooks"] = m


def _build_program():
    import concourse.bass as bass  # noqa: F401
    import concourse.mybir as mybir
    import concourse.tile as tile
    from concourse import bacc

    f16 = mybir.dt.float16
    f32 = mybir.dt.float32
    AF = mybir.ActivationFunctionType

    nc = bacc.Bacc("TRN2", target_bir_lowering=False, debug=False,
                   num_devices=NCORES)
    # register EPS as a const AP so activation(bias=EPS) works
    eps_t = nc.alloc_sbuf_tensor("const-eps", [128, 1], f32)
    nc.gpsimd.memset(eps_t.ap(), EPS)
    nc.const_aps.aps[(f32, EPS)] = eps_t.ap()
    nc.all_engine_barrier()

    def din(name, shape, dt=f16):
        return nc.dram_tensor(name, shape, dt, kind="ExternalInput").ap()

    xT = din("xT", [HID, S])              # transposed batch slice of x
    waq = din("waq", [HID, 256])          # q_a_w cols for this core's rank
    wakv = din("wakv", [HID, 384])        # kv_a_w cols: [ckv 256 | 0s 64 | kpe-perm 64]
    wqb = din("wqb", [QLORA, HPC * 128])  # per head: [nope64 | pe64-perm], ln folded
    wkn = din("wkn", [KVLORA, HPC * 64])  # per head: knope cols, ln folded
    wv = din("wv", [KVLORA, HPC * 128])   # per head: v cols, ln folded
    wo = din("wo", [HPC * VDIM, HID])     # o_w rows for this core's heads
    cosT = din("cosT", [128, S])          # rows 0:64 = 1, rows 64:128 = cos
    sinT = din("sinT", [128, S])          # rows 0:64 = 0, rows 64:128 = sin
    rotT = din("rotT", [128, 128])        # transposed rotate-half matrix
    maskT = din("maskT", [128, 4 * STW])  # causal diagonal masks j=0..3
    out = nc.dram_tensor("out", [S, HID], f16, kind="ExternalOutput").ap()

    W = HPC * 128
    qoff = [0, 128, 256, 384, 512, 640]

    from contextlib import ExitStack
    with tile.TileContext(nc) as tc, ExitStack() as ctx:
        const = ctx.enter_context(tc.tile_pool(name="const", bufs=1))
        waqp = ctx.enter_context(tc.tile_pool(name="waqp", bufs=16))
        wakvp = ctx.enter_context(tc.tile_pool(name="wakvp", bufs=16))
        xtp = ctx.enter_context(tc.tile_pool(name="xtp", bufs=32))
        xqap = ctx.enter_context(tc.tile_pool(name="xqap", bufs=6))
        qrawp = ctx.enter_context(tc.tile_pool(name="qrawp", bufs=2))
        krawp = ctx.enter_context(tc.tile_pool(name="krawp", bufs=2))
        sqp = ctx.enter_context(tc.tile_pool(name="sqp", bufs=3))
        tmpp = ctx.enter_context(tc.tile_pool(name="tmpp", bufs=4))
        smallp = ctx.enter_context(tc.tile_pool(name="smallp", bufs=4))
        bcp = ctx.enter_context(tc.tile_pool(name="bcp", bufs=3))
        csnp = ctx.enter_context(tc.tile_pool(name="csnp", bufs=4))
        qfp = ctx.enter_context(tc.tile_pool(name="qfp", bufs=8))
        persist = ctx.enter_context(tc.tile_pool(name="persist", bufs=HPC))
        ptp = ctx.enter_context(tc.tile_pool(name="ptp", bufs=4))
        aop = ctx.enter_context(tc.tile_pool(name="aop", bufs=8))
        obp = ctx.enter_context(tc.tile_pool(name="obp", bufs=3))
        ps_lat = ctx.enter_context(tc.tile_pool(name="ps_lat", bufs=3, space="PSUM"))
        ps_sc = ctx.enter_context(tc.tile_pool(name="ps_sc", bufs=2, space="PSUM"))
        ps_out = ctx.enter_context(tc.tile_pool(name="ps_out", bufs=1, space="PSUM"))
        ps_small = ctx.enter_context(tc.tile_pool(name="ps_small", bufs=2, space="PSUM"))

        # DRAM bounce tiles for the q_a feature-shard AllGather (per st)
        dram = ctx.enter_context(tc.tile_pool(name="dram", bufs=2, space="DRAM"))

        # ---- constants into SBUF ----
        sb_waq = [waqp.tile([128, 256], f16, tag="waq", name=f"waq{hc}")
                  for hc in range(16)]
        sb_wakv = [wakvp.tile([128, 384], f16, tag="wakv", name=f"wakv{hc}")
                   for hc in range(16)]
        sb_wqb = const.tile([128, 6 * W], f16, tag="wqb")
        sb_wkn = const.tile([128, 2 * HPC * 64], f16, tag="wkn")
        sb_wv = const.tile([128, 2 * W], f16, tag="wv")
        sb_wo = const.tile([128, 16 * STW], f16, tag="wo")
        sb_cos = const.tile([128, S], f16, tag="cos")
        sb_sin = const.tile([128, S], f16, tag="sin")
        sb_rot = const.tile([128, 128], f16, tag="rot")
        sb_mask = const.tile([128, 4 * STW], f16, tag="mask")
        sb_ones = const.tile([128, 1], f16, tag="ones")

        # preamble A: waq only; qa-own streams its own x chunks and frees them
        # immediately, so all 4 supertiles' q_a shards are computed and
        # gathered up front (x is re-streamed later for the kv path).
        for hc in range(16):
            nc.sync.dma_start(out=sb_waq[hc][:], in_=waq[hc * 128:(hc + 1) * 128, :])
        nc.vector.memset(sb_ones[:], 1.0)

        def qa_own(st):
            """This core's 2 q_a feature chunks for supertile st, streamed
            chunk-wise (both chunks accumulate concurrently so each x chunk
            tile is used twice then freed). Returns (qown, ppsum_sb)."""
            cols = slice(st * STW, (st + 1) * STW)
            qown = qrawp.tile([128, 2 * STW], f16, tag="qown")
            ppsum = ps_small.tile([1, STW], f32, tag="sums", name=f"pp{st}")
            psA = ps_lat.tile([128, STW], f32, tag="lat")
            psB = ps_lat.tile([128, STW], f32, tag="lat")
            for hc in range(16):
                xq = xqap.tile([128, STW], f16, tag="xqa", name=f"xq{st}_{hc}")
                nc.sync.dma_start(out=xq[:], in_=xT[hc * 128:(hc + 1) * 128, cols])
                nc.tensor.matmul(psA[:], sb_waq[hc][:, 0:128], xq[:],
                                 start=(hc == 0), stop=(hc == 15))
                nc.tensor.matmul(psB[:], sb_waq[hc][:, 128:256], xq[:],
                                 start=(hc == 0), stop=(hc == 15))
            for c, ps in ((0, psA), (1, psB)):
                nc.any.tensor_copy(qown[:, c * STW:(c + 1) * STW], ps[:])
                sq = sqp.tile([128, STW], f16, tag="sq")
                nc.scalar.activation(sq[:], ps[:], AF.Square)
                nc.tensor.matmul(ppsum[:], sb_ones[:, :], sq[:],
                                 start=(c == 0), stop=(c == 1))
            ppsum_sb = smallp.tile([1, STW], f16, tag="smallh", name=f"pps{st}")
            nc.any.tensor_copy(ppsum_sb[:], ppsum[:])
            return qown, ppsum_sb

        def gather(sts):
            """AllGather the q_a shards for supertiles `sts` across the
            4-core group. Per rank: 260 rows per st (2x128 features + sumsq
            row + pad). Returns shard_out with rank r / st index i at rows
            [260*(len(sts)*r + i), ...)."""
            n = len(sts)
            shard_in = dram.tile([n * 260, STW], f16, tag=f"shin{n}")
            shard_out = dram.tile([4 * n * 260, STW], f16, tag=f"shout{n}")
            for i, st in enumerate(sts):
                qown, ppsum_sb = qa_own(st)
                base = i * 260
                nc.sync.dma_start(out=shard_in[base:base + 128, :],
                                  in_=qown[:, 0:STW])
                nc.sync.dma_start(out=shard_in[base + 128:base + 256, :],
                                  in_=qown[:, STW:2 * STW])
                nc.sync.dma_start(out=shard_in[base + 256:base + 257, :],
                                  in_=ppsum_sb[:])
            nc.gpsimd.collective_compute(
                "AllGather",
                mybir.AluOpType.bypass,
                replica_groups=[[0, 1, 2, 3], [4, 5, 6, 7]],
                ins=[shard_in.opt()],
                outs=[shard_out.opt()],
            )
            return {st: (shard_out, n, i) for i, st in enumerate(sts)}

        shard_outs = {}
        shard_outs.update(gather([0]))
        # early x/wakv for the kv path of st0 (shadow work under AG0)
        xt0 = [xtp.tile([128, STW], f16, tag="xt", name=f"xt0_{hc}")
               for hc in range(16)]
        for hc in range(16):
            nc.sync.dma_start(out=xt0[hc][:], in_=xT[hc * 128:(hc + 1) * 128, 0:STW])
            nc.sync.dma_start(out=sb_wakv[hc][:], in_=wakv[hc * 128:(hc + 1) * 128, :])
        shard_outs.update(gather([1]))
        shard_outs.update(gather([2, 3]))

        # preamble B: remaining weights (gathers already in flight)
        for c in range(6):
            nc.sync.dma_start(out=sb_wqb[:QCH[c], c * W:(c + 1) * W],
                              in_=wqb[qoff[c]:qoff[c] + QCH[c], :])
        for c in range(2):
            nc.sync.dma_start(out=sb_wkn[:, c * HPC * 64:(c + 1) * HPC * 64],
                              in_=wkn[c * 128:(c + 1) * 128, :])
            nc.sync.dma_start(out=sb_wv[:, c * W:(c + 1) * W],
                              in_=wv[c * 128:(c + 1) * 128, :])
        nc.sync.dma_start(out=sb_cos[:], in_=cosT[:])
        nc.sync.dma_start(out=sb_sin[:], in_=sinT[:])
        nc.sync.dma_start(out=sb_rot[:], in_=rotT[:])
        nc.sync.dma_start(out=sb_mask[:], in_=maskT[:])
        for h in range(HPC):
            for hcn in range(4):
                nc.sync.dma_start(
                    out=sb_wo[:, (h * 4 + hcn) * STW:(h * 4 + hcn + 1) * STW],
                    in_=wo[h * 128:(h + 1) * 128, hcn * STW:(hcn + 1) * STW])

        # persistent per-head K/V (all supertiles)
        kfT = [persist.tile([128, S], f16, tag="kf", name=f"kfT{h}") for h in range(HPC)]
        VT = [persist.tile([128, S], f16, tag="vh", name=f"VT{h}") for h in range(HPC)]

        xt_cur = xt0
        for st in range(NST):
            cols = slice(st * STW, (st + 1) * STW)
            # prefetch next supertile's x
            if st < NST - 1:
                ncols = slice((st + 1) * STW, (st + 2) * STW)
                xt_nxt = [xtp.tile([128, STW], f16, tag="xt", name=f"xt{st+1}_{hc}")
                          for hc in range(16)]
                for hc in range(16):
                    nc.sync.dma_start(out=xt_nxt[hc][:],
                                      in_=xT[hc * 128:(hc + 1) * 128, ncols])
            xt = xt_cur
            shard_out, shn, shi = shard_outs[st]
            sums1 = ps_small.tile([128, STW], f32, tag="sums", name=f"s1_{st}")

            kraw = krawp.tile([128, 2 * STW], f16, tag="kraw")
            for c in range(2):
                ps = ps_lat.tile([128, STW], f32, tag="lat")
                for hc in range(16):
                    nc.tensor.matmul(
                        ps[:],
                        sb_wakv[hc][:, c * 128:(c + 1) * 128],
                        xt[hc][:],
                        start=(hc == 0), stop=(hc == 15))
                nc.any.tensor_copy(kraw[:, c * STW:(c + 1) * STW], ps[:])
                sq = sqp.tile([128, STW], f16, tag="sq")
                nc.scalar.activation(sq[:], ps[:], AF.Square)
                nc.tensor.matmul(sums1[32:33, :], sb_ones[:, :], sq[:],
                                 start=(c == 0), stop=(c == 1))
            ps = ps_lat.tile([128, STW], f32, tag="lat")
            for hc in range(16):
                nc.tensor.matmul(
                    ps[:],
                    sb_wakv[hc][:, 256:384],
                    xt[hc][:],
                    start=(hc == 0), stop=(hc == 15))
            kperaw = tmpp.tile([128, STW], f16, tag="tmp", name=f"kperaw{st}")
            nc.any.tensor_copy(kperaw[:], ps[:])

            # ===== P1b: unpack gathered q_a latents (ranks 0-2 are real;
            # rank 3's features are zero padding and are skipped) =====
            qraw = qrawp.tile([128, 6 * STW], f16, tag="qraw")
            for r in range(3):
                base = 260 * (shn * r + shi)
                for c2 in range(2):
                    blk = 2 * r + c2
                    nc.sync.dma_start(
                        out=qraw[:, blk * STW:(blk + 1) * STW],
                        in_=shard_out[base + 128 * c2:base + 128 * (c2 + 1), :])
            psrows = smallp.tile([3, STW], f16, tag="smallh", name=f"psr{st}")
            for r in range(3):
                base = 260 * (shn * r + shi)
                nc.sync.dma_start(out=psrows[r:r + 1, :],
                                  in_=shard_out[base + 256:base + 257, :])
            nc.tensor.matmul(sums1[0:1, :], sb_ones[0:3, :], psrows[:],
                             start=True, stop=True)

            # q norm chain: rstd = exp(-0.5*ln(meansq+eps)); ln+exp+square+copy
            # share one ACT table set (no table swaps vs Sqrt)
            stdq = smallp.tile([1, STW], f32, tag="small", name=f"stdq{st}")
            nc.scalar.activation(stdq[:], sums1[0:1, :], AF.Ln,
                                 bias=EPS, scale=1.0 / QLORA)
            rstdq = smallp.tile([1, STW], f32, tag="small", name=f"rstdq{st}")
            nc.scalar.activation(rstdq[:], stdq[:], AF.Exp, scale=-0.5)
            rstdq_b = bcp.tile([128, STW], f32, tag="bc", name=f"rqb{st}")
            nc.gpsimd.partition_broadcast(rstdq_b[:], rstdq[:])
            csp = csnp.tile([128, STW], f16, tag="csn", name=f"cs{st}")
            nc.vector.tensor_mul(csp[:], sb_cos[:, cols], rstdq_b[:])
            snp = csnp.tile([128, STW], f16, tag="csn", name=f"sn{st}")
            nc.vector.tensor_mul(snp[:], sb_sin[:, cols], rstdq_b[:])

            # k norm chain: normalize kraw in place
            stdk = smallp.tile([1, STW], f32, tag="small", name=f"stdk{st}")
            nc.scalar.activation(stdk[:], sums1[32:33, :], AF.Ln,
                                 bias=EPS, scale=1.0 / KVLORA)
            rstdk = smallp.tile([1, STW], f32, tag="small", name=f"rstdk{st}")
            nc.scalar.activation(rstdk[:], stdk[:], AF.Exp, scale=-0.5)
            rstdk_b = bcp.tile([128, STW], f32, tag="bc", name=f"rkb{st}")
            nc.gpsimd.partition_broadcast(rstdk_b[:], rstdk[:])
            for c in range(2):
                nc.vector.tensor_mul(kraw[:, c * STW:(c + 1) * STW],
                                     kraw[:, c * STW:(c + 1) * STW], rstdk_b[:])

            # shared k_pe rope (rows 64:128), written into kfT[0] then copied
            rps = ps_lat.tile([128, STW], f32, tag="lat")
            nc.tensor.matmul(rps[:], sb_rot[:], kperaw[:], start=True, stop=True)
            t1k = tmpp.tile([128, STW], f16, tag="tmp", name=f"t1k{st}")
            nc.vector.tensor_mul(t1k[64:128, :], rps[64:128, :],
                                 sb_sin[64:128, cols])
            nc.vector.tensor_mul(kfT[0][64:128, cols], kperaw[64:128, :],
                                 sb_cos[64:128, cols])
            nc.vector.tensor_add(kfT[0][64:128, cols], kfT[0][64:128, cols],
                                 t1k[64:128, :])
            for h in range(1, HPC):
                nc.any.tensor_copy(kfT[h][64:128, cols], kfT[0][64:128, cols])

            # ===== P1 per head: q_b + rope, k_nope, V =====
            qf_st = []
            for h in range(HPC):
                psq = ps_lat.tile([128, STW], f32, tag="lat")
                for c in range(6):
                    nc.tensor.matmul(
                        psq[:],
                        sb_wqb[:QCH[c], c * W + h * 128:c * W + (h + 1) * 128],
                        qraw[:QCH[c], c * STW:(c + 1) * STW],
                        start=(c == 0), stop=(c == 5))
                qraw_h = tmpp.tile([128, STW], f16, tag="tmp", name=f"qr{st}_{h}")
                nc.any.tensor_copy(qraw_h[:], psq[:])
                rq = ps_lat.tile([128, STW], f32, tag="lat")
                nc.tensor.matmul(rq[:], sb_rot[:], qraw_h[:], start=True, stop=True)
                qf = qfp.tile([128, STW], f16, tag="qf", name=f"qf{st}_{h}")
                nc.vector.tensor_mul(qf[:], qraw_h[:], csp[:])
                t1q = tmpp.tile([128, STW], f16, tag="tmp", name=f"t1q{st}_{h}")
                nc.vector.tensor_mul(t1q[64:128, :], rq[64:128, :], snp[64:128, :])
                nc.vector.tensor_add(qf[64:128, :], qf[64:128, :], t1q[64:128, :])
                qf_st.append(qf)

                psk = ps_lat.tile([128, STW], f32, tag="lat")
                for c in range(2):
                    nc.tensor.matmul(
                        psk[:64, :],
                        sb_wkn[:, c * HPC * 64 + h * 64:c * HPC * 64 + (h + 1) * 64],
                        kraw[:, c * STW:(c + 1) * STW],
                        start=(c == 0), stop=(c == 1))
                nc.any.tensor_copy(kfT[h][0:64, cols], psk[:64, :])

                psv = ps_lat.tile([128, STW], f32, tag="lat")
                for tcn in range(4):
                    for c in range(2):
                        nc.tensor.matmul(
                            psv[:, tcn * VDIM:(tcn + 1) * VDIM],
                            kraw[:, c * STW + tcn * 128:c * STW + (tcn + 1) * 128],
                            sb_wv[:, c * W + h * 128:c * W + (h + 1) * 128],
                            start=(c == 0), stop=(c == 1))
                nc.any.tensor_copy(VT[h][:, cols], psv[:])

            # ===== P2: attention for qs = st =====
            nkc = 4 * st + 4
            aout_st = []
            for h in range(HPC):
                sums = ps_small.tile([1, STW], f32, tag="sums", name=f"s{st}_{h}")
                outp = ps_out.tile([128, STW], f32, tag="out")
                for kc in range(nkc):
                    # diagonal chunks: queries < 128*j are fully masked, skip
                    j = kc - 4 * st
                    q0 = 128 * j if j > 0 else 0
                    qsl = slice(q0, STW)
                    sc = ps_sc.tile([128, STW], f32, tag="sc")
                    nc.tensor.matmul(sc[:, qsl],
                                     kfT[h][:, kc * 128:(kc + 1) * 128],
                                     qf_st[h][:, qsl],
                                     start=True, stop=True)
                    pt = ptp.tile([128, STW], f16, tag="pt")
                    nc.scalar.activation(pt[:, qsl], sc[:, qsl], AF.Exp,
                                         scale=SCALE)
                    if j >= 0:
                        nc.vector.tensor_mul(
                            pt[:, qsl], pt[:, qsl],
                            sb_mask[:, j * STW + q0:(j + 1) * STW])
                    nc.tensor.matmul(sums[:, qsl], sb_ones[:, :], pt[:, qsl],
                                     start=(kc == 0), stop=(kc == nkc - 1),
                                     skip_group_check=True)
                    nc.tensor.matmul(outp[:, qsl],
                                     VT[h][:, kc * VDIM:(kc + 1) * VDIM],
                                     pt[:, qsl],
                                     start=(kc == 0), stop=(kc == nkc - 1),
                                     skip_group_check=True)
                rs = smallp.tile([1, STW], f32, tag="small", name=f"rs{st}_{h}")
                nc.vector.reciprocal_approx_fast(out=rs[:], in_=sums[:])
                rs_b = bcp.tile([128, STW], f32, tag="bc", name=f"rsb{st}_{h}")
                nc.gpsimd.partition_broadcast(rs_b[:], rs[:])
                ao = aop.tile([128, STW], f16, tag="ao", name=f"ao{st}_{h}")
                nc.vector.tensor_mul(ao[:], outp[:], rs_b[:])
                aout_st.append(ao)

            # ===== P3: o projection for this supertile's tokens =====
            for tl in range(4):
                tcn = 4 * st + tl
                for hcn in range(4):
                    pso = ps_lat.tile([128, STW], f32, tag="lat")
                    for h in range(HPC):
                        nc.tensor.matmul(
                            pso[:],
                            aout_st[h][:, tl * 128:(tl + 1) * 128],
                            sb_wo[:, (h * 4 + hcn) * STW:(h * 4 + hcn + 1) * STW],
                            start=(h == 0), stop=(h == HPC - 1))
                    ob = obp.tile([128, STW], f16, tag="ob")
                    nc.any.tensor_copy(ob[:], pso[:])
                    nc.sync.dma_start(
                        out=out[tcn * 128:(tcn + 1) * 128,
                                hcn * STW:(hcn + 1) * STW],
                        in_=ob[:])

            if st < NST - 1:
                xt_cur = xt_nxt

    nc.compile()
    return nc


def _host_prep(inputs):
    f16 = np.float16
    x = np.asarray(inputs["x"], np.float32)
    q_a_w = np.asarray(inputs["q_a_w"], np.float32)
    q_a_ln = np.asarray(inputs["q_a_ln_w"], np.float32)
    q_b_w = np.asarray(inputs["q_b_w"], np.float32)
    kv_a_w = np.asarray(inputs["kv_a_w"], np.float32)
    kv_a_ln = np.asarray(inputs["kv_a_ln_w"], np.float32)
    kv_b_w = np.asarray(inputs["kv_b_w"], np.float32)
    o_w = np.asarray(inputs["o_w"], np.float32)

    perm = np.concatenate([np.arange(0, ROPE, 2), np.arange(1, ROPE, 2)])
    q_b_f = q_b_w * q_a_ln[:, None]
    kv_b_f = kv_b_w * kv_a_ln[:, None]

    # kv_a padded: [ckv 256 | zeros 64 | kpe perm 64]
    wakv = np.concatenate(
        [kv_a_w[:, :KVLORA],
         np.zeros((HID, 64), np.float32),
         kv_a_w[:, KVLORA:][:, perm]], axis=1).astype(f16)
    # q_a feature shards: group rank r owns feature cols [256r, 256r+256),
    # zero-padded past QLORA (rank 3 is all padding)
    waq_shards = []
    for r in range(4):
        w = np.zeros((HID, 256), np.float32)
        lo = 256 * r
        hi = min(QLORA, lo + 256)
        if lo < QLORA:
            w[:, :hi - lo] = q_a_w[:, lo:hi]
        waq_shards.append(w.astype(f16))

    # rope tables (transposed [dim, pos])
    inv = 1.0 / (THETA ** (np.arange(0, ROPE, 2, dtype=np.float64) / ROPE))
    freqs = np.outer(np.arange(S, dtype=np.float64), inv)      # [S, 32]
    cos64 = np.concatenate([np.cos(freqs), np.cos(freqs)], -1).T  # [64, S]
    sin64 = np.concatenate([np.sin(freqs), np.sin(freqs)], -1).T
    cosT = np.concatenate([np.ones((64, S)), cos64], 0).astype(f16)
    sinT = np.concatenate([np.zeros((64, S)), sin64], 0).astype(f16)

    # rotate-half matrix: out = ROT @ xp, nonzero only on rows/cols 64:128
    R64 = np.zeros((64, 64), np.float32)
    for j in range(32):
        R64[j, 32 + j] = -1.0
        R64[32 + j, j] = 1.0
    ROT = np.zeros((128, 128), np.float32)
    ROT[64:, 64:] = R64
    rotT = ROT.T.astype(f16)

    # diagonal causal masks: mask_j[k, q] = k <= q - 128*j
    k_i = np.arange(128)[:, None]
    q_i = np.arange(STW)[None, :]
    maskT = np.concatenate(
        [(k_i <= q_i - 128 * j).astype(f16) for j in range(4)], axis=1)

    in_maps = []
    for core in range(NCORES):
        b = core // 4
        heads = [HPC * (core % 4) + i for i in range(HPC)]
        wqb = np.concatenate(
            [np.concatenate(
                [q_b_f[:, h * QHEAD:h * QHEAD + NOPE],
                 q_b_f[:, h * QHEAD + NOPE:(h + 1) * QHEAD][:, perm]], 1)
             for h in heads], axis=1).astype(f16)
        wkn = np.concatenate(
            [kv_b_f[:, h * (NOPE + VDIM):h * (NOPE + VDIM) + NOPE]
             for h in heads], axis=1).astype(f16)
        wv = np.concatenate(
            [kv_b_f[:, h * (NOPE + VDIM) + NOPE:(h + 1) * (NOPE + VDIM)]
             for h in heads], axis=1).astype(f16)
        wo = np.concatenate(
            [o_w[h * VDIM:(h + 1) * VDIM, :] for h in heads], axis=0).astype(f16)
        in_maps.append({
            "xT": np.ascontiguousarray(x[b].T).astype(f16),
            "waq": waq_shards[core % 4], "wakv": wakv, "wqb": wqb,
            "wkn": wkn, "wv": wv,
            "wo": wo, "cosT": cosT, "sinT": sinT, "rotT": rotT,
            "maskT": maskT,
        })
    return in_maps


def kernel(**inputs):
    global _PROGRAM
    _ensure_axon_hooks_shim()
    from concourse.bass_utils import run_bass_kernel_spmd

    if _PROGRAM is None:
        _PROGRAM = _build_program()
    in_maps = _host_prep(inputs)
    res = run_bass_kernel_spmd(_PROGRAM, in_maps, list(range(NCORES)))
    out = np.zeros((B, S, HID), np.float32)
    for core in range(NCORES):
        out[core // 4] += res.results[core]["out"].astype(np.float32)
    return out


# revision 20
# speedup vs baseline: 1.3061x; 1.3061x over previous
"""DeepSeek MLA head — Trainium2 Bass kernel, 8 NeuronCores. v2.

Sharding: 8 cores = 2 batches x 4 cores. Each core owns one batch and 4 of
the 16 heads; latent projections replicated within each batch's 4 cores;
each core emits a partial o_proj output [S, HID] (f16) which the host sums.

v2 vs v1 (652us):
- Supertile-merged schedule: P1 (latents/qkv), P2 (attention qs=st), P3
  (o-proj token chunks of st) interleave per supertile so TensorE always has
  independent matmul work and HAM stays warm.
- DVE `reciprocal` (3.3us, single-lane) replaced by reciprocal_approx_fast.
- Broadcast matmuls + ScalarE copies replaced by gpsimd.partition_broadcast.
- RMSNorm of q folded into the rope cos/sin tables (per-supertile), so q_b
  matmuls never wait on the norm chain.
- PSUM->SBUF copies on nc.any (scheduler balances ACT/DVE).
- Output stored f16 (halved DMA), host accumulates in f32.
"""
import sys
import types

sys.path.insert(0, "/opt/trn_rl_repo")

import numpy as np

B, S, HID, NH = 2, 2048, 2048, 16
ROPE, NOPE, VDIM = 64, 64, 128
QHEAD, QLORA, KVLORA = 128, 682, 256
THETA = 128000.0
SCALE = 1.0 / float(np.sqrt(128.0))
EPS = 1e-6
HPC = 4              # heads per core
NCORES = 8
QCH = [128, 128, 128, 128, 128, 42]   # qlora partition chunks
NST = 4              # 512-token supertiles per batch
STW = 512

_PROGRAM = None


def _ensure_axon_hooks_shim():
    if "antenv.axon_hooks" in sys.modules:
        return
    try:
        from trn_agent_boot.trn_boot import _ntff_profile_via_ctypes
        hook = _ntff_profile_via_ctypes("/opt/axon/libaxon_pjrt.so")
    except Exception:
        hook = None
    m = types.ModuleType("antenv.axon_hooks")
    m.get_axon_ntff_profile_hook = lambda: hook
    m.set_axon_ntff_profile_hook = lambda h: None
    sys.modules["antenv.axon_hooks"] = m


def _build_program():
    import concourse.bass as bass  # noqa: F401
    import concourse.mybir as mybir
    import concourse.tile as tile
    from concourse import bacc

    f16 = mybir.dt.float16
    f32 = mybir.dt.float32
    AF = mybir.ActivationFunctionType

    nc = bacc.Bacc("TRN2", target_bir_lowering=False, debug=False,
                   num_devices=NCORES)
    # register EPS as a const AP so activation(bias=EPS) works
    eps_t = nc.alloc_sbuf_tensor("const-eps", [128, 1], f32)
    nc.gpsimd.memset(eps_t.ap(), EPS)
    nc.const_aps.aps[(f32, EPS)] = eps_t.ap()
    nc.all_engine_barrier()

    def din(name, shape, dt=f16):
        return nc.dram_tensor(name, shape, dt, kind="ExternalInput").ap()

    xT = din("xT", [HID, S])              # transposed batch slice of x
    waq = din("waq", [HID, QLORA])        # q_a_w
    wakv = din("wakv", [HID, 384])        # kv_a_w cols: [ckv 256 | 0s 64 | kpe-perm 64]
    wqb = din("wqb", [QLORA, HPC * 128])  # per head: [nope64 | pe64-perm], ln folded
    wkn = din("wkn", [KVLORA, HPC * 64])  # per head: knope cols, ln folded
    wv = din("wv", [KVLORA, HPC * 128])   # per head: v cols, ln folded
    wo = din("wo", [HPC * VDIM, HID])     # o_w rows for this core's heads
    cosT = din("cosT", [128, S])          # rows 0:64 = 1, rows 64:128 = cos
    sinT = din("sinT", [128, S])          # rows 0:64 = 0, rows 64:128 = sin
    rotT = din("rotT", [128, 128])        # transposed rotate-half matrix
    maskT = din("maskT", [128, 4 * STW])  # causal diagonal masks j=0..3
    out = nc.dram_tensor("out", [S, HID], f16, kind="ExternalOutput").ap()

    W = HPC * 128
    qoff = [0, 128, 256, 384, 512, 640]

    from contextlib import ExitStack
    with tile.TileContext(nc) as tc, ExitStack() as ctx:
        const = ctx.enter_context(tc.tile_pool(name="const", bufs=1))
        waqp = ctx.enter_context(tc.tile_pool(name="waqp", bufs=16))
        wakvp = ctx.enter_context(tc.tile_pool(name="wakvp", bufs=16))
        xtp = ctx.enter_context(tc.tile_pool(name="xtp", bufs=32))
        qrawp = ctx.enter_context(tc.tile_pool(name="qrawp", bufs=2))
        krawp = ctx.enter_context(tc.tile_pool(name="krawp", bufs=2))
        sqp = ctx.enter_context(tc.tile_pool(name="sqp", bufs=3))
        tmpp = ctx.enter_context(tc.tile_pool(name="tmpp", bufs=4))
        smallp = ctx.enter_context(tc.tile_pool(name="smallp", bufs=4))
        bcp = ctx.enter_context(tc.tile_pool(name="bcp", bufs=3))
        csnp = ctx.enter_context(tc.tile_pool(name="csnp", bufs=4))
        qfp = ctx.enter_context(tc.tile_pool(name="qfp", bufs=8))
        persist = ctx.enter_context(tc.tile_pool(name="persist", bufs=HPC))
        ptp = ctx.enter_context(tc.tile_pool(name="ptp", bufs=4))
        aop = ctx.enter_context(tc.tile_pool(name="aop", bufs=8))
        obp = ctx.enter_context(tc.tile_pool(name="obp", bufs=3))
        ps_lat = ctx.enter_context(tc.tile_pool(name="ps_lat", bufs=3, space="PSUM"))
        ps_sc = ctx.enter_context(tc.tile_pool(name="ps_sc", bufs=2, space="PSUM"))
        ps_out = ctx.enter_context(tc.tile_pool(name="ps_out", bufs=1, space="PSUM"))
        ps_small = ctx.enter_context(tc.tile_pool(name="ps_small", bufs=2, space="PSUM"))

        # ---- constants into SBUF ----
        sb_waq = [waqp.tile([128, QLORA], f16, tag="waq", name=f"waq{hc}")
                  for hc in range(16)]
        sb_wakv = [wakvp.tile([128, 384], f16, tag="wakv", name=f"wakv{hc}")
                   for hc in range(16)]
        sb_wqb = const.tile([128, 6 * W], f16, tag="wqb")
        sb_wkn = const.tile([128, 2 * HPC * 64], f16, tag="wkn")
        sb_wv = const.tile([128, 2 * W], f16, tag="wv")
        sb_wo = const.tile([128, 16 * STW], f16, tag="wo")
        sb_cos = const.tile([128, S], f16, tag="cos")
        sb_sin = const.tile([128, S], f16, tag="sin")
        sb_rot = const.tile([128, 128], f16, tag="rot")
        sb_mask = const.tile([128, 4 * STW], f16, tag="mask")
        sb_ones = const.tile([128, 1], f16, tag="ones")

        # x chunks for st0 + weights
        xt0 = [xtp.tile([128, STW], f16, tag="xt", name=f"xt0_{hc}")
               for hc in range(16)]
        for hc in range(16):
            nc.sync.dma_start(out=sb_waq[hc][:], in_=waq[hc * 128:(hc + 1) * 128, :])
            nc.sync.dma_start(out=xt0[hc][:], in_=xT[hc * 128:(hc + 1) * 128, 0:STW])
        nc.vector.memset(sb_ones[:], 1.0)

        for hc in range(16):
            nc.sync.dma_start(out=sb_wakv[hc][:], in_=wakv[hc * 128:(hc + 1) * 128, :])
        for c in range(6):
            nc.sync.dma_start(out=sb_wqb[:QCH[c], c * W:(c + 1) * W],
                              in_=wqb[qoff[c]:qoff[c] + QCH[c], :])
        for c in range(2):
            nc.sync.dma_start(out=sb_wkn[:, c * HPC * 64:(c + 1) * HPC * 64],
                              in_=wkn[c * 128:(c + 1) * 128, :])
            nc.sync.dma_start(out=sb_wv[:, c * W:(c + 1) * W],
                              in_=wv[c * 128:(c + 1) * 128, :])
        nc.sync.dma_start(out=sb_cos[:], in_=cosT[:])
        nc.sync.dma_start(out=sb_sin[:], in_=sinT[:])
        nc.sync.dma_start(out=sb_rot[:], in_=rotT[:])
        nc.sync.dma_start(out=sb_mask[:], in_=maskT[:])
        for h in range(HPC):
            for hcn in range(4):
                nc.sync.dma_start(
                    out=sb_wo[:, (h * 4 + hcn) * STW:(h * 4 + hcn + 1) * STW],
                    in_=wo[h * 128:(h + 1) * 128, hcn * STW:(hcn + 1) * STW])

        # persistent per-head K/V (all supertiles)
        kfT = [persist.tile([128, S], f16, tag="kf", name=f"kfT{h}") for h in range(HPC)]
        VT = [persist.tile([128, S], f16, tag="vh", name=f"VT{h}") for h in range(HPC)]

        xt_cur = xt0
        for st in range(NST):
            cols = slice(st * STW, (st + 1) * STW)
            # prefetch next supertile's x
            if st < NST - 1:
                ncols = slice((st + 1) * STW, (st + 2) * STW)
                xt_nxt = [xtp.tile([128, STW], f16, tag="xt", name=f"xt{st+1}_{hc}")
                          for hc in range(16)]
                for hc in range(16):
                    nc.sync.dma_start(out=xt_nxt[hc][:],
                                      in_=xT[hc * 128:(hc + 1) * 128, ncols])
            xt = xt_cur
            sums1 = ps_small.tile([128, STW], f32, tag="sums", name=f"s1_{st}")
            qraw = qrawp.tile([128, 6 * STW], f16, tag="qraw")
            for c in range(6):
                ps = ps_lat.tile([128, STW], f32, tag="lat")
                for hc in range(16):
                    nc.tensor.matmul(
                        ps[:QCH[c], :],
                        sb_waq[hc][:, qoff[c]:qoff[c] + QCH[c]],
                        xt[hc][:],
                        start=(hc == 0), stop=(hc == 15))
                nc.any.tensor_copy(qraw[:QCH[c], c * STW:(c + 1) * STW],
                                   ps[:QCH[c], :])
                sq = sqp.tile([128, STW], f16, tag="sq")
                nc.scalar.activation(sq[:QCH[c], :], ps[:QCH[c], :], AF.Square)
                nc.tensor.matmul(sums1[0:1, :], sb_ones[:QCH[c], :], sq[:QCH[c], :],
                                 start=(c == 0), stop=(c == 5))

            kraw = krawp.tile([128, 2 * STW], f16, tag="kraw")
            for c in range(2):
                ps = ps_lat.tile([128, STW], f32, tag="lat")
                for hc in range(16):
                    nc.tensor.matmul(
                        ps[:],
                        sb_wakv[hc][:, c * 128:(c + 1) * 128],
                        xt[hc][:],
                        start=(hc == 0), stop=(hc == 15))
                nc.any.tensor_copy(kraw[:, c * STW:(c + 1) * STW], ps[:])
                sq = sqp.tile([128, STW], f16, tag="sq")
                nc.scalar.activation(sq[:], ps[:], AF.Square)
                nc.tensor.matmul(sums1[32:33, :], sb_ones[:, :], sq[:],
                                 start=(c == 0), stop=(c == 1))
            ps = ps_lat.tile([128, STW], f32, tag="lat")
            for hc in range(16):
                nc.tensor.matmul(
                    ps[:],
                    sb_wakv[hc][:, 256:384],
                    xt[hc][:],
                    start=(hc == 0), stop=(hc == 15))
            kperaw = tmpp.tile([128, STW], f16, tag="tmp", name=f"kperaw{st}")
            nc.any.tensor_copy(kperaw[:], ps[:])

            # q norm chain: rstd = exp(-0.5*ln(meansq+eps)); ln+exp+square+copy
            # share one ACT table set (no table swaps vs Sqrt)
            stdq = smallp.tile([1, STW], f32, tag="small", name=f"stdq{st}")
            nc.scalar.activation(stdq[:], sums1[0:1, :], AF.Ln,
                                 bias=EPS, scale=1.0 / QLORA)
            rstdq = smallp.tile([1, STW], f32, tag="small", name=f"rstdq{st}")
            nc.scalar.activation(rstdq[:], stdq[:], AF.Exp, scale=-0.5)
            rstdq_b = bcp.tile([128, STW], f32, tag="bc", name=f"rqb{st}")
            nc.gpsimd.partition_broadcast(rstdq_b[:], rstdq[:])
            csp = csnp.tile([128, STW], f16, tag="csn", name=f"cs{st}")
            nc.vector.tensor_mul(csp[:], sb_cos[:, cols], rstdq_b[:])
            snp = csnp.tile([128, STW], f16, tag="csn", name=f"sn{st}")
            nc.vector.tensor_mul(snp[:], sb_sin[:, cols], rstdq_b[:])

            # k norm chain: normalize kraw in place
            stdk = smallp.tile([1, STW], f32, tag="small", name=f"stdk{st}")
            nc.scalar.activation(stdk[:], sums1[32:33, :], AF.Ln,
                                 bias=EPS, scale=1.0 / KVLORA)
            rstdk = smallp.tile([1, STW], f32, tag="small", name=f"rstdk{st}")
            nc.scalar.activation(rstdk[:], stdk[:], AF.Exp, scale=-0.5)
            rstdk_b = bcp.tile([128, STW], f32, tag="bc", name=f"rkb{st}")
            nc.gpsimd.partition_broadcast(rstdk_b[:], rstdk[:])
            for c in range(2):
                nc.vector.tensor_mul(kraw[:, c * STW:(c + 1) * STW],
                                     kraw[:, c * STW:(c + 1) * STW], rstdk_b[:])

            # shared k_pe rope (rows 64:128), written into kfT[0] then copied
            rps = ps_lat.tile([128, STW], f32, tag="lat")
            nc.tensor.matmul(rps[:], sb_rot[:], kperaw[:], start=True, stop=True)
            t1k = tmpp.tile([128, STW], f16, tag="tmp", name=f"t1k{st}")
            nc.vector.tensor_mul(t1k[64:128, :], rps[64:128, :],
                                 sb_sin[64:128, cols])
            nc.vector.tensor_mul(kfT[0][64:128, cols], kperaw[64:128, :],
                                 sb_cos[64:128, cols])
            nc.vector.tensor_add(kfT[0][64:128, cols], kfT[0][64:128, cols],
                                 t1k[64:128, :])
            for h in range(1, HPC):
                nc.any.tensor_copy(kfT[h][64:128, cols], kfT[0][64:128, cols])

            # ===== P1 per head: q_b + rope, k_nope, V =====
            qf_st = []
            for h in range(HPC):
                psq = ps_lat.tile([128, STW], f32, tag="lat")
                for c in range(6):
                    nc.tensor.matmul(
                        psq[:],
                        sb_wqb[:QCH[c], c * W + h * 128:c * W + (h + 1) * 128],
                        qraw[:QCH[c], c * STW:(c + 1) * STW],
                        start=(c == 0), stop=(c == 5))
                qraw_h = tmpp.tile([128, STW], f16, tag="tmp", name=f"qr{st}_{h}")
                nc.any.tensor_copy(qraw_h[:], psq[:])
                rq = ps_lat.tile([128, STW], f32, tag="lat")
                nc.tensor.matmul(rq[:], sb_rot[:], qraw_h[:], start=True, stop=True)
                qf = qfp.tile([128, STW], f16, tag="qf", name=f"qf{st}_{h}")
                nc.vector.tensor_mul(qf[:], qraw_h[:], csp[:])
                t1q = tmpp.tile([128, STW], f16, tag="tmp", name=f"t1q{st}_{h}")
                nc.vector.tensor_mul(t1q[64:128, :], rq[64:128, :], snp[64:128, :])
                nc.vector.tensor_add(qf[64:128, :], qf[64:128, :], t1q[64:128, :])
                qf_st.append(qf)

                psk = ps_lat.tile([128, STW], f32, tag="lat")
                for c in range(2):
                    nc.tensor.matmul(
                        psk[:64, :],
                        sb_wkn[:, c * HPC * 64 + h * 64:c * HPC * 64 + (h + 1) * 64],
                        kraw[:, c * STW:(c + 1) * STW],
                        start=(c == 0), stop=(c == 1))
                nc.any.tensor_copy(kfT[h][0:64, cols], psk[:64, :])

                psv = ps_lat.tile([128, STW], f32, tag="lat")
                for tcn in range(4):
                    for c in range(2):
                        nc.tensor.matmul(
                            psv[:, tcn * VDIM:(tcn + 1) * VDIM],
                            kraw[:, c * STW + tcn * 128:c * STW + (tcn + 1) * 128],
                            sb_wv[:, c * W + h * 128:c * W + (h + 1) * 128],
                            start=(c == 0), stop=(c == 1))
                nc.any.tensor_copy(VT[h][:, cols], psv[:])

            # ===== P2: attention for qs = st =====
            nkc = 4 * st + 4
            aout_st = []
            for h in range(HPC):
                sums = ps_small.tile([1, STW], f32, tag="sums", name=f"s{st}_{h}")
                outp = ps_out.tile([128, STW], f32, tag="out")
                for kc in range(nkc):
                    # diagonal chunks: queries < 128*j are fully masked, skip
                    j = kc - 4 * st
                    q0 = 128 * j if j > 0 else 0
                    qsl = slice(q0, STW)
                    sc = ps_sc.tile([128, STW], f32, tag="sc")
                    nc.tensor.matmul(sc[:, qsl],
                                     kfT[h][:, kc * 128:(kc + 1) * 128],
                                     qf_st[h][:, qsl],
                                     start=True, stop=True)
                    pt = ptp.tile([128, STW], f16, tag="pt")
                    nc.scalar.activation(pt[:, qsl], sc[:, qsl], AF.Exp,
                                         scale=SCALE)
                    if j >= 0:
                        nc.vector.tensor_mul(
                            pt[:, qsl], pt[:, qsl],
                            sb_mask[:, j * STW + q0:(j + 1) * STW])
                    nc.tensor.matmul(sums[:, qsl], sb_ones[:, :], pt[:, qsl],
                                     start=(kc == 0), stop=(kc == nkc - 1),
                                     skip_group_check=True)
                    nc.tensor.matmul(outp[:, qsl],
                                     VT[h][:, kc * VDIM:(kc + 1) * VDIM],
                                     pt[:, qsl],
                                     start=(kc == 0), stop=(kc == nkc - 1),
                                     skip_group_check=True)
                rs = smallp.tile([1, STW], f32, tag="small", name=f"rs{st}_{h}")
                nc.vector.reciprocal_approx_fast(out=rs[:], in_=sums[:])
                rs_b = bcp.tile([128, STW], f32, tag="bc", name=f"rsb{st}_{h}")
                nc.gpsimd.partition_broadcast(rs_b[:], rs[:])
                ao = aop.tile([128, STW], f16, tag="ao", name=f"ao{st}_{h}")
                nc.vector.tensor_mul(ao[:], outp[:], rs_b[:])
                aout_st.append(ao)

            # ===== P3: o projection for this supertile's tokens =====
            for tl in range(4):
                tcn = 4 * st + tl
                for hcn in range(4):
                    pso = ps_lat.tile([128, STW], f32, tag="lat")
                    for h in range(HPC):
                        nc.tensor.matmul(
                            pso[:],
                            aout_st[h][:, tl * 128:(tl + 1) * 128],
                            sb_wo[:, (h * 4 + hcn) * STW:(h * 4 + hcn + 1) * STW],
                            start=(h == 0), stop=(h == HPC - 1))
                    ob = obp.tile([128, STW], f16, tag="ob")
                    nc.any.tensor_copy(ob[:], pso[:])
                    nc.sync.dma_start(
                        out=out[tcn * 128:(tcn + 1) * 128,
                                hcn * STW:(hcn + 1) * STW],
                        in_=ob[:])

            if st < NST - 1:
                xt_cur = xt_nxt

    nc.compile()
    return nc


def _host_prep(inputs):
    f16 = np.float16
    x = np.asarray(inputs["x"], np.float32)
    q_a_w = np.asarray(inputs["q_a_w"], np.float32)
    q_a_ln = np.asarray(inputs["q_a_ln_w"], np.float32)
    q_b_w = np.asarray(inputs["q_b_w"], np.float32)
    kv_a_w = np.asarray(inputs["kv_a_w"], np.float32)
    kv_a_ln = np.asarray(inputs["kv_a_ln_w"], np.float32)
    kv_b_w = np.asarray(inputs["kv_b_w"], np.float32)
    o_w = np.asarray(inputs["o_w"], np.float32)

    perm = np.concatenate([np.arange(0, ROPE, 2), np.arange(1, ROPE, 2)])
    q_b_f = q_b_w * q_a_ln[:, None]
    kv_b_f = kv_b_w * kv_a_ln[:, None]

    # kv_a padded: [ckv 256 | zeros 64 | kpe perm 64]
    wakv = np.concatenate(
        [kv_a_w[:, :KVLORA],
         np.zeros((HID, 64), np.float32),
         kv_a_w[:, KVLORA:][:, perm]], axis=1).astype(f16)
    waq = q_a_w.astype(f16)

    # rope tables (transposed [dim, pos])
    inv = 1.0 / (THETA ** (np.arange(0, ROPE, 2, dtype=np.float64) / ROPE))
    freqs = np.outer(np.arange(S, dtype=np.float64), inv)      # [S, 32]
    cos64 = np.concatenate([np.cos(freqs), np.cos(freqs)], -1).T  # [64, S]
    sin64 = np.concatenate([np.sin(freqs), np.sin(freqs)], -1).T
    cosT = np.concatenate([np.ones((64, S)), cos64], 0).astype(f16)
    sinT = np.concatenate([np.zeros((64, S)), sin64], 0).astype(f16)

    # rotate-half matrix: out = ROT @ xp, nonzero only on rows/cols 64:128
    R64 = np.zeros((64, 64), np.float32)
    for j in range(32):
        R64[j, 32 + j] = -1.0
        R64[32 + j, j] = 1.0
    ROT = np.zeros((128, 128), np.float32)
    ROT[64:, 64:] = R64
    rotT = ROT.T.astype(f16)

    # diagonal causal masks: mask_j[k, q] = k <= q - 128*j
    k_i = np.arange(128)[:, None]
    q_i = np.arange(STW)[None, :]
    maskT = np.concatenate(
        [(k_i <= q_i - 128 * j).astype(f16) for j in range(4)], axis=1)

    in_maps = []
    for core in range(NCORES):
        b = core // 4
        heads = [HPC * (core % 4) + i for i in range(HPC)]
        wqb = np.concatenate(
            [np.concatenate(
                [q_b_f[:, h * QHEAD:h * QHEAD + NOPE],
                 q_b_f[:, h * QHEAD + NOPE:(h + 1) * QHEAD][:, perm]], 1)
             for h in heads], axis=1).astype(f16)
        wkn = np.concatenate(
            [kv_b_f[:, h * (NOPE + VDIM):h * (NOPE + VDIM) + NOPE]
             for h in heads], axis=1).astype(f16)
        wv = np.concatenate(
            [kv_b_f[:, h * (NOPE + VDIM) + NOPE:(h + 1) * (NOPE + VDIM)]
             for h in heads], axis=1).astype(f16)
        wo = np.concatenate(
            [o_w[h * VDIM:(h + 1) * VDIM, :] for h in heads], axis=0).astype(f16)
        in_maps.append({
            "xT": np.ascontiguousarray(x[b].T).astype(f16),
            "waq": waq, "wakv": wakv, "wqb": wqb,
            "wkn": wkn, "wv": wv,
            "wo": wo, "cosT": cosT, "sinT": sinT, "rotT": rotT,
            "maskT": maskT,
        })
    return in_maps


def kernel(**inputs):
    global _PROGRAM
    _ensure_axon_hooks_shim()
    from concourse.bass_utils import run_bass_kernel_spmd

    if _PROGRAM is None:
        _PROGRAM = _build_program()
    in_maps = _host_prep(inputs)
    res = run_bass_kernel_spmd(_PROGRAM, in_maps, list(range(NCORES)))
    out = np.zeros((B, S, HID), np.float32)
    for core in range(NCORES):
        out[core // 4] += res.results[core]["out"].astype(np.float32)
    return out
